# revision 1
# baseline (speedup 1.0000x reference)
# Trainium2 Bass kernel for nn_Net_4861902979707
#
# Computation (per sample, B = 4194304):
#   X [B, 3, 3] -> 3 pairwise Euclidean distances d = [d01, d02, d12]
#   h1 = elu(d @ W1.T + b1); h2 = elu(h1 @ W2.T + b2); y = h2 @ W3.T + b3
#
# Strategy: pure data parallel over 8 NeuronCores (batch split), sample-major
# layout on chip: tiles of [128 partitions, T samples]. Distances partly on
# DVE (pairwise diffs as contiguous-write "plane" ops; d12 = d02 - d01),
# squares on DVE/ACT, then the TensorEngine does every linear reduction as
# diagonal-matrix matmuls accumulated in PSUM fp32: coord sums (identity
# lhsT), all three MLP layers (W*I lhsT). ELU is elu(z)+1 = relu(z+b) +
# exp(-relu(-(z+b))) on ACT (biases fused into the activation); the +1
# shift is absorbed into the next layer's bias on the host (b' = b - W @ 1).
import os as _os
import numpy as np

B = 4194304
N_CORES = 8
B_CORE = B // N_CORES          # 524288
P = 128                        # partitions
T = int(_os.environ.get("TSZ", "512"))
TILE = P * T
N_TILES = B_CORE // TILE

# intermediate dtype: "bf16" (fast) or "fp32" (accurate)
COMPUTE_DT = "bf16"

XCAST = _os.environ.get("XCAST", "0") == "1"    # X delivered as bf16 (host cast)
SQ_ACT = int(_os.environ.get("SQ_ACT", "0"))    # pairs squared on ACT (0-3)
ELU_MODE = _os.environ.get("ELU_MODE", "dve")   # act | dve
STAGE = _os.environ.get("STAGE", "full")        # full | dma | dist
BUFS_X = int(_os.environ.get("BUFS_X", "4"))
BUFS_W = int(_os.environ.get("BUFS_W", "3"))
BUFS_M = int(_os.environ.get("BUFS_M", "3"))

_CACHE = {}


def _split_sync_waits(nc, mybir, limit=1):
    """This walrus build rejects instructions carrying more than ~1 sem wait
    ("Too many sync wait commands"). Hoist excess waits onto NoOp carrier
    instructions (same engine, immediately before) — engine program order
    preserves the blocking semantics."""
    n_split = 0
    for f in nc.m.functions:
        for b in f.blocks:
            lst = b.instructions
            out = []
            changed = False
            for inst in lst:
                si = inst.sync_info
                if si is not None and len(si.on_wait) > limit:
                    waits = list(si.on_wait)
                    extra, keep = waits[:-limit], waits[-limit:]
                    for wi, w in enumerate(extra):
                        nop = mybir.InstNoOp(
                            name=f"wsplit-{inst.name}-{wi}")
                        nop.engine = inst.engine
                        nop.sync_info = mybir.SyncInfo(
                            on_wait=[w], on_update=[])
                        out.append(nop)
                        n_split += 1
                    inst.sync_info = type(si)(
                        on_wait=keep, on_update=list(si.on_update))
                    changed = True
                out.append(inst)
            if changed:
                b.instructions = out
    return n_split


# WD diag-matrix indices (each a [128,128] lhsT); weights stored as
# bf16 hi+lo pairs so the PE path keeps ~fp32 weight precision
def _iWD_I():
    return 0
def _iWD_W1(k, j, p):
    return 1 + 2 * (3 * k + j) + p
def _iWD_W2(m, j, p):
    return 13 + 2 * (2 * m + j) + p
def _iWD_W3(j, p):
    return 21 + 2 * j + p
N_WD = 25

# WB scalar indices: b1[k]=k, b2'[m]=2+m, b3'=4, -b1[k]=5+k, -b2'[m]=7+m
def _ib1(k):
    return k
def _ib2(m):
    return 2 + m
_IB3 = 4
def _inb1(k):
    return 5 + k
def _inb2(m):
    return 7 + m
N_WB = 9


def _build(dt_name, reps=1, bench_small=False):
    import concourse.bass as bass
    import concourse.tile as tile
    import concourse.mybir as mybir

    f32 = mybir.dt.float32
    bf16 = mybir.dt.bfloat16
    dt = bf16 if dt_name == "bf16" else f32
    Alu = mybir.AluOpType
    Act = mybir.ActivationFunctionType

    nc = bass.Bass()
    BC = TILE if bench_small else B_CORE
    X = nc.dram_tensor("X", [BC, 9], dt if XCAST else f32,
                       kind="ExternalInput")
    WB = nc.dram_tensor("WB", [N_WB], f32, kind="ExternalInput")
    WD = nc.dram_tensor("WD", [N_WD, P, P], dt, kind="ExternalInput")
    Y = nc.dram_tensor("Y", [BC, 1], f32, kind="ExternalOutput")

    PAIRS = [(0, 1), (0, 2)]  # pair 2 (1,2) comes from d02 - d01

    with tile.TileContext(nc) as tc:
        with (
            tc.tile_pool(name="singles", bufs=1) as singles,
            tc.tile_pool(name="xin", bufs=BUFS_X) as xin,
            tc.tile_pool(name="work", bufs=BUFS_W) as work,
            tc.tile_pool(name="mlp", bufs=BUFS_M) as mlp,
            tc.tile_pool(name="yout", bufs=3) as yout,
            tc.tile_pool(name="psum", bufs=1, space="PSUM") as psum,
        ):
            # broadcast bias scalars to all partitions; load diag matrices
            wb = singles.tile([P, N_WB], f32)
            nc.gpsimd.dma_start(
                out=wb[:],
                in_=bass.AP(tensor=WB[:].tensor, offset=0,
                            ap=[[0, P], [1, N_WB]]))
            wd = singles.tile([P, N_WD, P], dt)
            nc.sync.dma_start(
                out=wd[:],
                in_=bass.AP(tensor=WD[:].tensor, offset=0,
                            ap=[[P, P], [P * P, N_WD], [1, P]]))

            def ws(i):  # [P,1] bias scalar AP
                return wb[:, i:i + 1]

            def diag(i):  # [128,128] lhsT AP
                return wd[:, i, :]

            # reps>1 wraps the whole body in a For_i loop (benchmarking only)
            _loop = tc.For_i(0, reps) if reps != 1 else None
            if _loop is not None:
                _loop.__enter__()

            for ti in range(N_TILES):
                src = 0 if bench_small else ti
                xr = X[src * TILE:(src + 1) * TILE, :].rearrange(
                    "(p s) d -> p s d", p=P)
                xt = xin.tile([P, T, 9], dt if XCAST else f32)
                nc.sync.dma_start(out=xt[:], in_=xr)

                yr = Y[src * TILE:(src + 1) * TILE, :].rearrange(
                    "(p s) d -> p (s d)", p=P)

                if STAGE == "dma":
                    yt = yout.tile([P, T], f32)
                    nc.scalar.activation(yt, xt[:, :, 0], Act.Copy)
                    nc.sync.dma_start(out=yr, in_=yt[:])
                    continue

                # pairwise diffs -> planes [P, 9, T]; contiguous writes
                diff = work.tile([P, 9, T], dt)
                for pi, (i, j) in enumerate(PAIRS):
                    for c in range(3):
                        nc.vector.tensor_sub(
                            diff[:, 3 * pi + c, :],
                            xt[:, :, 3 * i + c],
                            xt[:, :, 3 * j + c],
                        )
                # d12 = d02 - d01 (contiguous bf16, 2x)
                nc.vector.tensor_sub(
                    diff[:, 6:9, :], diff[:, 3:6, :], diff[:, 0:3, :])

                # squares in place, per pair (SQ_ACT of them on ACT)
                sq = diff
                for pi in range(3):
                    pl = diff[:, 3 * pi:3 * pi + 3, :]
                    if pi < SQ_ACT:
                        nc.scalar.activation(pl, pl, Act.Square)
                    else:
                        nc.vector.tensor_mul(pl, pl, pl)

                # coord sums on PE: q_pi = I@sq0 + I@sq1 + I@sq2 (PSUM fp32)
                qs = []
                for pi in range(3):
                    q = psum.tile([P, T], f32, tag=f"q{pi}")
                    for c in range(3):
                        nc.tensor.matmul(
                            q[:], diag(_iWD_I()), sq[:, 3 * pi + c, :],
                            start=(c == 0), stop=(c == 2))
                    qs.append(q)

                # distances (ACT sqrt, PSUM -> SBUF bf16)
                dist = work.tile([P, 3, T], dt)
                for pi in range(3):
                    nc.scalar.activation(dist[:, pi, :], qs[pi][:], Act.Sqrt)

                if STAGE == "dist":
                    yt = yout.tile([P, T], f32)
                    nc.scalar.activation(yt, dist[:, 0, :], Act.Copy)
                    nc.sync.dma_start(out=yr, in_=yt[:])
                    continue

                def elu(z_psum, ib, inb, tag):
                    """h = relu(z+b) + exp(min(z+b, 0)) from PSUM z."""
                    r = mlp.tile([P, T], dt, tag=f"r_{tag}")
                    nc.scalar.activation(
                        r, z_psum[:], Act.Relu, bias=ws(ib), scale=1.0)
                    e = mlp.tile([P, T], dt, tag=f"e_{tag}")
                    if ELU_MODE == "act":
                        rm = mlp.tile([P, T], dt, tag=f"rm_{tag}")
                        nc.scalar.activation(
                            rm, z_psum[:], Act.Relu, bias=ws(inb), scale=-1.0)
                        nc.scalar.activation(e, rm, Act.Exp, scale=-1.0)
                    else:
                        m = mlp.tile([P, T], dt, tag=f"rm_{tag}")
                        nc.vector.tensor_scalar(
                            out=m, in0=z_psum[:], scalar1=ws(ib),
                            scalar2=0.0, op0=Alu.add, op1=Alu.min)
                        nc.scalar.activation(e, m, Act.Exp)
                    h = mlp.tile([P, T], dt, tag=f"h_{tag}")
                    nc.vector.tensor_add(h, r, e)
                    return h

                # L1 on PE: z_k = sum_j W1[k,j]*I @ d_j  (PSUM fp32)
                h1 = []
                for k in range(2):
                    z = psum.tile([P, T], f32, tag=f"z1_{k}")
                    for j in range(3):
                        for p_ in range(2):
                            nc.tensor.matmul(
                                z[:], diag(_iWD_W1(k, j, p_)), dist[:, j, :],
                                start=(j == 0 and p_ == 0),
                                stop=(j == 2 and p_ == 1))
                    h1.append(elu(z, _ib1(k), _inb1(k), f"1{k}"))

                # L2
                h2 = []
                for m_ in range(2):
                    z = psum.tile([P, T], f32, tag=f"z2_{m_}")
                    for j in range(2):
                        for p_ in range(2):
                            nc.tensor.matmul(
                                z[:], diag(_iWD_W2(m_, j, p_)), h1[j][:],
                                start=(j == 0 and p_ == 0),
                                stop=(j == 1 and p_ == 1))
                    h2.append(elu(z, _ib2(m_), _inb2(m_), f"2{m_}"))

                # L3
                yz = psum.tile([P, T], f32, tag="yz")
                for j in range(2):
                    for p_ in range(2):
                        nc.tensor.matmul(
                            yz[:], diag(_iWD_W3(j, p_)), h2[j][:],
                            start=(j == 0 and p_ == 0),
                            stop=(j == 1 and p_ == 1))
                yt = yout.tile([P, T], f32)
                nc.scalar.activation(
                    yt, yz[:], Act.Identity, bias=ws(_IB3), scale=1.0)
                nc.sync.dma_start(out=yr, in_=yt[:])

            if _loop is not None:
                _loop.__exit__(None, None, None)

    _split_sync_waits(nc, mybir, limit=1)
    return nc


def _pack_weights(W1, b1, W2, b2, W3, b3):
    import ml_dtypes
    W1 = np.asarray(W1, np.float32); b1 = np.asarray(b1, np.float32)
    W2 = np.asarray(W2, np.float32); b2 = np.asarray(b2, np.float32)
    W3 = np.asarray(W3, np.float32); b3 = np.asarray(b3, np.float32)
    wb = np.empty(N_WB, np.float32)
    b2a = b2 - W2.sum(axis=1)            # absorb elu(+1) shift
    b3a = b3 - W3.sum(axis=1)
    wb[0:2] = b1
    wb[2:4] = b2a
    wb[4] = b3a[0]
    wb[5:7] = -b1
    wb[7:9] = -b2a

    dt = ml_dtypes.bfloat16 if COMPUTE_DT == "bf16" else np.float32
    eye = np.eye(P, dtype=np.float32)

    def hilo(w):
        hi = np.float32(np.asarray(w, dt).astype(np.float32))
        lo = np.float32(w) - hi
        return hi, lo

    wdf = np.empty((N_WD, P, P), np.float32)
    wdf[_iWD_I()] = eye
    for k in range(2):
        for j in range(3):
            hi, lo = hilo(W1[k, j])
            wdf[_iWD_W1(k, j, 0)] = eye * hi
            wdf[_iWD_W1(k, j, 1)] = eye * lo
    for m in range(2):
        for j in range(2):
            hi, lo = hilo(W2[m, j])
            wdf[_iWD_W2(m, j, 0)] = eye * hi
            wdf[_iWD_W2(m, j, 1)] = eye * lo
    for j in range(2):
        hi, lo = hilo(W3[0, j])
        wdf[_iWD_W3(j, 0)] = eye * hi
        wdf[_iWD_W3(j, 1)] = eye * lo
    return wb, wdf.astype(dt)


LAST_RESULTS = None  # BassKernelResults of the most recent run (for test.py)


def kernel(X, W1, b1, W2, b2, W3, b3):
    from concourse.bass_utils import run_bass_kernel_spmd
    import ml_dtypes
    global LAST_RESULTS

    X = np.ascontiguousarray(np.asarray(X, np.float32).reshape(B, 9))
    if XCAST:
        X = X.astype(ml_dtypes.bfloat16 if COMPUTE_DT == "bf16"
                     else np.float32)
    wb, wd = _pack_weights(W1, b1, W2, b2, W3, b3)

    key = (COMPUTE_DT, 1)
    if key not in _CACHE:
        _CACHE[key] = _build(COMPUTE_DT)
    nc = _CACHE[key]

    in_maps = [
        {"X": X[c * B_CORE:(c + 1) * B_CORE], "WB": wb, "WD": wd}
        for c in range(N_CORES)
    ]
    res = run_bass_kernel_spmd(nc, in_maps, core_ids=list(range(N_CORES)))
    LAST_RESULTS = res
    out = np.concatenate([res.results[c]["Y"] for c in range(N_CORES)], axis=0)
    return out.reshape(B, 1)



# revision 12
# speedup vs baseline: 1.1310x; 1.1310x over previous
# Trainium2 Bass kernel for nn_Net_4861902979707
#
# Computation (per sample, B = 4194304):
#   X [B, 3, 3] -> 3 pairwise Euclidean distances d = [d01, d02, d12]
#   h1 = elu(d @ W1.T + b1); h2 = elu(h1 @ W2.T + b2); y = h2 @ W3.T + b3
#
# Data parallel over 8 NeuronCores. Host ships X as bf16 in plane-major
# tile layout [tile, partition, 9 planes, T] so every vector op is a
# contiguous wide op. Per tile [128, T]:
#   DVE : d01/d02 plane subs, squares (5 planes), elu tails
#   Pool: d12 = d02 - d01 and its squares (SBUF only; GPSIMD can't PSUM)
#   PE  : coord sums (identity diag), layer bias init (I @ bias-plane,
#         start=True), all MLP layers (diag matmuls)
#   ACT : sqrt x3, one square plane, exp x2, y copy+bias
# Tiles are processed in GROUPS so the activation table (sqrt set vs exp
# set) switches at group boundaries, not every tile (a load is ~1.3us).
# ELU identity (exact): with v = z + b (bias pre-added in PSUM by PE),
#   elu(v) = max(min(exp(v) - 1, 0), v)
# realized as ACT e = exp(v); DVE ec = (e min 1) add -1; DVE h = max(ec, v).
import os as _os
import numpy as np

B = 4194304
N_CORES = 8
B_CORE = B // N_CORES          # 524288
P = 128                        # partitions
T = int(_os.environ.get("TSZ", "512"))
TILE = P * T
N_TILES = B_CORE // TILE
TPG = int(_os.environ.get("TPG", "4"))     # tiles per act-table group
SQ_DVE = int(_os.environ.get("SQ_DVE", "5"))  # sq planes on DVE (of 0..5)
BUFS_X = int(_os.environ.get("BUFS_X", "3"))
BUFS_W = int(_os.environ.get("BUFS_W", "2"))
BUFS_M = int(_os.environ.get("BUFS_M", "2"))

_CACHE = {}


def _split_sync_waits(nc, mybir, limit=1):
    """This walrus build rejects instructions carrying more than ~1 sem wait
    ("Too many sync wait commands"). Hoist excess waits onto NoOp carrier
    instructions (same engine, immediately before) — engine program order
    preserves the blocking semantics."""
    n_split = 0
    for f in nc.m.functions:
        for b in f.blocks:
            lst = b.instructions
            out = []
            changed = False
            for inst in lst:
                si = inst.sync_info
                if si is not None and len(si.on_wait) > limit:
                    waits = list(si.on_wait)
                    extra, keep = waits[:-limit], waits[-limit:]
                    for wi, w in enumerate(extra):
                        nop = mybir.InstNoOp(
                            name=f"wsplit-{inst.name}-{wi}")
                        nop.engine = inst.engine
                        nop.sync_info = mybir.SyncInfo(
                            on_wait=[w], on_update=[])
                        out.append(nop)
                        n_split += 1
                    inst.sync_info = type(si)(
                        on_wait=keep, on_update=list(si.on_update))
                    changed = True
                out.append(inst)
            if changed:
                b.instructions = out
    return n_split


# WD diag-matrix indices (each a [128,128] bf16 lhsT)
def _iWD_I():
    return 0
def _iWD_W1(k, j):
    return 1 + 3 * k + j
def _iWD_W2(m, j):
    return 7 + 2 * m + j
def _iWD_W3(j):
    return 11 + j
N_WD = 13

_IB3 = 0
N_WB = 1


def _build(reps=1, bench_small=False):
    import concourse.bass as bass
    import concourse.tile as tile
    import concourse.mybir as mybir

    f32 = mybir.dt.float32
    bf16 = mybir.dt.bfloat16
    Alu = mybir.AluOpType
    Act = mybir.ActivationFunctionType

    nc = bass.Bass()
    NROW = P if bench_small else N_TILES * P
    X = nc.dram_tensor("X", [NROW, 9 * T], bf16, kind="ExternalInput")
    WB = nc.dram_tensor("WB", [N_WB], f32, kind="ExternalInput")
    WD = nc.dram_tensor("WD", [N_WD, P, P], bf16, kind="ExternalInput")
    BV = nc.dram_tensor("BV", [4 * T], bf16, kind="ExternalInput")
    YROWS = TILE if bench_small else B_CORE
    Y = nc.dram_tensor("Y", [YROWS, 1], f32, kind="ExternalOutput")

    with tile.TileContext(nc) as tc:
        with (
            tc.tile_pool(name="singles", bufs=1) as singles,
            tc.tile_pool(name="xin", bufs=BUFS_X) as xin,
            tc.tile_pool(name="work", bufs=BUFS_W) as work,
            tc.tile_pool(name="dring", bufs=TPG + 1) as dring,
            tc.tile_pool(name="mlp", bufs=BUFS_M) as mlp,
            tc.tile_pool(name="yout", bufs=2) as yout,
            tc.tile_pool(name="psq", bufs=1, space="PSUM") as psq,
            tc.tile_pool(name="pz", bufs=1, space="PSUM") as pz,
        ):
            # bias scalar (b3), diag matrices, bias planes (b1|b2 over 2T each)
            wb = singles.tile([P, N_WB], f32)
            nc.gpsimd.dma_start(
                out=wb[:],
                in_=bass.AP(tensor=WB[:].tensor, offset=0,
                            ap=[[0, P], [1, N_WB]]))
            wd = singles.tile([P, N_WD, P], bf16)
            nc.sync.dma_start(
                out=wd[:],
                in_=bass.AP(tensor=WD[:].tensor, offset=0,
                            ap=[[P, P], [P * P, N_WD], [1, P]]))
            bv = singles.tile([P, 4 * T], bf16)
            nc.sync.dma_start(
                out=bv[:],
                in_=bass.AP(tensor=BV[:].tensor, offset=0,
                            ap=[[0, P], [1, 4 * T]]))

            def ws(i):  # [P,1] scalar AP
                return wb[:, i:i + 1]

            def diag(i):  # [128,128] lhsT AP
                return wd[:, i, :]

            def emit_a(ti):
                """distances: DMA in, subs, squares, coord sums, sqrt."""
                src = 0 if bench_small else ti
                xt = xin.tile([P, 9 * T], bf16)
                nc.sync.dma_start(out=xt[:], in_=X[src * P:(src + 1) * P, :])

                diff = work.tile([P, 9, T], bf16, tag="diff")
                dv = diff[:].rearrange("p a b -> p (a b)")
                nc.vector.tensor_sub(
                    dv[:, 0:3 * T], xt[:, 0:3 * T], xt[:, 3 * T:6 * T])
                nc.vector.tensor_sub(
                    dv[:, 3 * T:6 * T], xt[:, 0:3 * T], xt[:, 6 * T:9 * T])
                nc.gpsimd.tensor_sub(
                    dv[:, 6 * T:9 * T], dv[:, 3 * T:6 * T], dv[:, 0:3 * T])

                sq = work.tile([P, 9, T], bf16, tag="sq")
                sv = sq[:].rearrange("p a b -> p (a b)")
                nd = SQ_DVE * T
                nc.vector.tensor_mul(sv[:, 0:nd], dv[:, 0:nd], dv[:, 0:nd])
                if SQ_DVE < 6:
                    nc.scalar.activation(
                        sv[:, nd:6 * T], dv[:, nd:6 * T], Act.Square)
                nc.gpsimd.tensor_mul(
                    sv[:, 6 * T:9 * T], dv[:, 6 * T:9 * T],
                    dv[:, 6 * T:9 * T])

                dt_ = dring.tile([P, 3, T], bf16)
                for pi in range(3):
                    q = psq.tile([P, T], f32, tag=f"q{pi}")
                    for c in range(3):
                        nc.tensor.matmul(
                            q[:], diag(_iWD_I()), sq[:, 3 * pi + c, :],
                            start=(c == 0), stop=(c == 2))
                    nc.scalar.activation(dt_[:, pi, :], q[:], Act.Sqrt)
                return dt_

            def elu_tail(z, tag):
                """z [P,2,T] PSUM holds v = W@x + b. Returns h = elu(v):
                ACT e = exp(v); DVE ec = (e min 1) - 1; DVE h = max(ec, v)."""
                zf = z[:].rearrange("p a b -> p (a b)")
                e = mlp.tile([P, 2, T], bf16, tag=f"e{tag}")
                ef = e[:].rearrange("p a b -> p (a b)")
                nc.scalar.activation(ef, zf, Act.Exp)
                nc.vector.tensor_scalar(
                    out=ef, in0=ef, scalar1=1.0, scalar2=-1.0,
                    op0=Alu.min, op1=Alu.add)
                h = mlp.tile([P, 2, T], bf16, tag=f"h{tag}")
                nc.vector.tensor_tensor(
                    out=h[:].rearrange("p a b -> p (a b)"), in0=ef, in1=zf,
                    op=Alu.max)
                return h

            def emit_b(ti, dt_):
                """MLP from distances d [P,3,T]; writes Y tile."""
                src = 0 if bench_small else ti
                z1 = pz.tile([P, 2, T], f32, tag="z1")
                for k in range(2):
                    nc.tensor.matmul(
                        z1[:, k, :], diag(_iWD_I()), bv[:, k * T:(k + 1) * T],
                        start=True, stop=False)
                    for j in range(3):
                        nc.tensor.matmul(
                            z1[:, k, :], diag(_iWD_W1(k, j)), dt_[:, j, :],
                            start=False, stop=(j == 2))
                h1 = elu_tail(z1, "1")

                z2 = pz.tile([P, 2, T], f32, tag="z2")
                for m in range(2):
                    nc.tensor.matmul(
                        z2[:, m, :], diag(_iWD_I()),
                        bv[:, (2 + m) * T:(3 + m) * T],
                        start=True, stop=False)
                    for j in range(2):
                        nc.tensor.matmul(
                            z2[:, m, :], diag(_iWD_W2(m, j)), h1[:, j, :],
                            start=False, stop=(j == 1))
                h2 = elu_tail(z2, "2")

                yz = pz.tile([P, T], f32, tag="yz")
                for j in range(2):
                    nc.tensor.matmul(
                        yz[:], diag(_iWD_W3(j)), h2[:, j, :],
                        start=(j == 0), stop=(j == 1))

                yr = Y[src * TILE:(src + 1) * TILE, :].rearrange(
                    "(p s) d -> p (s d)", p=P)
                yo = yout.tile([P, T], f32)
                nc.scalar.activation(
                    yo, yz[:], Act.Identity, bias=ws(_IB3), scale=1.0)
                nc.sync.dma_start(out=yr, in_=yo[:])

            # reps>1 wraps the whole body in a For_i loop (benchmarking only)
            _loop = tc.For_i(0, reps) if reps != 1 else None
            if _loop is not None:
                _loop.__enter__()

            n_groups = (N_TILES + TPG - 1) // TPG
            for g in range(n_groups):
                tis = range(g * TPG, min((g + 1) * TPG, N_TILES))
                dts = [emit_a(ti) for ti in tis]
                for ti, dt_ in zip(tis, dts):
                    emit_b(ti, dt_)

            if _loop is not None:
                _loop.__exit__(None, None, None)

    _split_sync_waits(nc, mybir, limit=1)
    return nc


def _pack_weights(W1, b1, W2, b2, W3, b3):
    import ml_dtypes
    W1 = np.asarray(W1, np.float32); b1 = np.asarray(b1, np.float32)
    W2 = np.asarray(W2, np.float32); b2 = np.asarray(b2, np.float32)
    W3 = np.asarray(W3, np.float32); b3 = np.asarray(b3, np.float32)
    wb = np.empty(N_WB, np.float32)
    wb[_IB3] = b3[0]

    bf = ml_dtypes.bfloat16
    eye = np.eye(P, dtype=np.float32)
    wdf = np.empty((N_WD, P, P), np.float32)
    wdf[_iWD_I()] = eye
    for k in range(2):
        for j in range(3):
            wdf[_iWD_W1(k, j)] = eye * W1[k, j]
    for m in range(2):
        for j in range(2):
            wdf[_iWD_W2(m, j)] = eye * W2[m, j]
    for j in range(2):
        wdf[_iWD_W3(j)] = eye * W3[0, j]
    bvec = np.empty(4 * T, np.float32)
    bvec[0:T] = b1[0]
    bvec[T:2 * T] = b1[1]
    bvec[2 * T:3 * T] = b2[0]
    bvec[3 * T:4 * T] = b2[1]
    return wb, wdf.astype(bf), bvec.astype(bf)


def _pack_x(X):
    """[B,3,3] fp32 -> per-core bf16 plane-major [N_TILES*P, 9*T]."""
    import ml_dtypes
    Xb = np.asarray(X, np.float32).reshape(B, 9).astype(ml_dtypes.bfloat16)
    out = []
    for c in range(N_CORES):
        xc = Xb[c * B_CORE:(c + 1) * B_CORE]
        xc = xc.reshape(N_TILES, P, T, 9).transpose(0, 1, 3, 2)
        out.append(np.ascontiguousarray(xc).reshape(N_TILES * P, 9 * T))
    return out


LAST_RESULTS = None  # BassKernelResults of the most recent run (for test.py)


def kernel(X, W1, b1, W2, b2, W3, b3):
    from concourse.bass_utils import run_bass_kernel_spmd
    global LAST_RESULTS

    xs = _pack_x(X)
    wb, wd, bvec = _pack_weights(W1, b1, W2, b2, W3, b3)

    if "nc" not in _CACHE:
        _CACHE["nc"] = _build()
    nc = _CACHE["nc"]

    in_maps = [
        {"X": xs[c], "WB": wb, "WD": wd, "BV": bvec}
        for c in range(N_CORES)
    ]
    res = run_bass_kernel_spmd(nc, in_maps, core_ids=list(range(N_CORES)))
    LAST_RESULTS = res
    out = np.concatenate([res.results[c]["Y"] for c in range(N_CORES)], axis=0)
    return out.reshape(B, 1)


# revision 28
# speedup vs baseline: 1.4038x; 1.2413x over previous
# Trainium2 Bass kernel for nn_Net_4861902979707
#
# Computation (per sample, B = 4194304):
#   X [B, 3, 3] -> 3 pairwise Euclidean distances d = [d01, d02, d12]
#   h1 = elu(d @ W1.T + b1); h2 = elu(h1 @ W2.T + b2); y = h2 @ W3.T + b3
#
# Data parallel over 8 NeuronCores. Host ships X as bf16 in plane-major
# tile layout [tile, partition, 9 planes, T] so every vector op is a
# contiguous wide op. Per tile [128, T]:
#   DVE : d01/d02 plane subs, squares (5 planes), elu tails
#   Pool: d12 = d02 - d01 and its squares (SBUF only; GPSIMD can't PSUM)
#   PE  : coord sums (identity diag), layer bias init (I @ bias-plane,
#         start=True), all MLP layers (diag matmuls)
#   ACT : sqrt x3, one square plane, exp x2, y copy+bias
# Tiles are processed in GROUPS so the activation table (sqrt set vs exp
# set) switches at group boundaries, not every tile (a load is ~1.3us).
# ELU identity (exact): with v = z + b (bias pre-added in PSUM by PE),
#   elu(v) = max(min(exp(v) - 1, 0), v)
# realized as ACT e = exp(v); DVE ec = (e min 1) add -1; DVE h = max(ec, v).
import os as _os
import numpy as np

B = 4194304
N_CORES = 8
B_CORE = B // N_CORES          # 524288
P = 128                        # partitions
T = int(_os.environ.get("TSZ", "512"))
TILE = P * T
N_TILES = B_CORE // TILE
TPG = int(_os.environ.get("TPG", "4"))     # tiles per act-table group
SQ_DVE = int(_os.environ.get("SQ_DVE", "5"))  # sq planes 0..5 on DVE (rest ACT)
D12 = _os.environ.get("D12", "pool")       # d12 sub: pool | dve
SQ12 = _os.environ.get("SQ12", "pool")     # sq of d12: pool | dve | act
SUMS_DVE = int(_os.environ.get("SUMS_DVE", "0"))  # pairs summed on DVE (0-3)
ELU = _os.environ.get("ELU", "max")        # max | reludve
YOUT = _os.environ.get("YOUT", "act")      # act | dve
NOSQRT = _os.environ.get("NOSQRT", "0") == "1"  # sqrt(q)=exp(0.5*ln q);
# keeps every ACT func in one table set (no ~1.3us table reloads)
BUFS_X = int(_os.environ.get("BUFS_X", "3"))
BUFS_W = int(_os.environ.get("BUFS_W", "2"))
BUFS_M = int(_os.environ.get("BUFS_M", "2"))
PZ_BUFS = int(_os.environ.get("PZ_BUFS", "1"))

_CACHE = {}


def _split_sync_waits(nc, mybir, limit=1):
    """This walrus build rejects instructions carrying more than ~1 sem wait
    ("Too many sync wait commands"). Hoist excess waits onto NoOp carrier
    instructions (same engine, immediately before) — engine program order
    preserves the blocking semantics."""
    n_split = 0
    for f in nc.m.functions:
        for b in f.blocks:
            lst = b.instructions
            out = []
            changed = False
            for inst in lst:
                si = inst.sync_info
                if si is not None and len(si.on_wait) > limit:
                    waits = list(si.on_wait)
                    extra, keep = waits[:-limit], waits[-limit:]
                    for wi, w in enumerate(extra):
                        nop = mybir.InstNoOp(
                            name=f"wsplit-{inst.name}-{wi}")
                        nop.engine = inst.engine
                        nop.sync_info = mybir.SyncInfo(
                            on_wait=[w], on_update=[])
                        out.append(nop)
                        n_split += 1
                    inst.sync_info = type(si)(
                        on_wait=keep, on_update=list(si.on_update))
                    changed = True
                out.append(inst)
            if changed:
                b.instructions = out
    return n_split


# WD diag-matrix indices (each a [128,128] bf16 lhsT)
def _iWD_I():
    return 0
def _iWD_W1(k, j):
    return 1 + 3 * k + j
def _iWD_W2(m, j):
    return 7 + 2 * m + j
def _iWD_W3(j):
    return 11 + j
N_WD = 13

# WB scalar indices
_IB3 = 0        # b3 (max form)
_IB3X = 1       # b3 - W3@1 (relu form)
def _ib1(k):
    return 2 + k
def _ib2x(m):
    return 4 + m
def _ieb1(k):
    return 6 + k
def _ieb2x(m):
    return 8 + m
N_WB = 10


def _build(reps=1, bench_small=False, unroll=False):
    import concourse.bass as bass
    import concourse.tile as tile
    import concourse.mybir as mybir

    f32 = mybir.dt.float32
    bf16 = mybir.dt.bfloat16
    Alu = mybir.AluOpType
    Act = mybir.ActivationFunctionType

    nc = bass.Bass()
    NROW = P if bench_small else N_TILES * P
    X = nc.dram_tensor("X", [NROW, 9 * T], bf16, kind="ExternalInput")
    WB = nc.dram_tensor("WB", [N_WB], f32, kind="ExternalInput")
    WD = nc.dram_tensor("WD", [N_WD, P, P], bf16, kind="ExternalInput")
    BV = nc.dram_tensor("BV", [4 * T], bf16, kind="ExternalInput")
    YROWS = TILE if bench_small else B_CORE
    Y = nc.dram_tensor("Y", [YROWS, 1], f32, kind="ExternalOutput")

    with tile.TileContext(nc) as tc:
        with (
            tc.tile_pool(name="singles", bufs=1) as singles,
            tc.tile_pool(name="xin", bufs=BUFS_X) as xin,
            tc.tile_pool(name="work", bufs=BUFS_W) as work,
            tc.tile_pool(name="dring", bufs=min(TPG + 1, 4)) as dring,
            tc.tile_pool(name="lgp", bufs=3) as lgp,
            tc.tile_pool(name="mlp", bufs=BUFS_M) as mlp,
            tc.tile_pool(name="yout", bufs=2) as yout,
            tc.tile_pool(name="psq", bufs=1, space="PSUM") as psq,
            tc.tile_pool(name="pz1", bufs=PZ_BUFS, space="PSUM") as pz1,
            tc.tile_pool(name="pz", bufs=1, space="PSUM") as pz,
        ):
            # bias scalar (b3), diag matrices, bias planes (b1|b2 over 2T each)
            wb = singles.tile([P, N_WB], f32)
            nc.gpsimd.dma_start(
                out=wb[:],
                in_=bass.AP(tensor=WB[:].tensor, offset=0,
                            ap=[[0, P], [1, N_WB]]))
            wd = singles.tile([P, N_WD, P], bf16)
            nc.sync.dma_start(
                out=wd[:],
                in_=bass.AP(tensor=WD[:].tensor, offset=0,
                            ap=[[P, P], [P * P, N_WD], [1, P]]))
            bv = singles.tile([P, 4 * T], bf16)
            nc.sync.dma_start(
                out=bv[:],
                in_=bass.AP(tensor=BV[:].tensor, offset=0,
                            ap=[[0, P], [1, 4 * T]]))

            def ws(i):  # [P,1] scalar AP
                return wb[:, i:i + 1]

            def diag(i):  # [128,128] lhsT AP
                return wd[:, i, :]

            def emit_a(ti):
                """distances: DMA in, subs, squares, coord sums, sqrt."""
                src = 0 if bench_small else ti
                xt = xin.tile([P, 9 * T], bf16)
                nc.sync.dma_start(out=xt[:], in_=X[src * P:(src + 1) * P, :])

                diff = work.tile([P, 9, T], bf16, tag="diff")
                dv = diff[:].rearrange("p a b -> p (a b)")
                nc.vector.tensor_sub(
                    dv[:, 0:3 * T], xt[:, 0:3 * T], xt[:, 3 * T:6 * T])
                nc.vector.tensor_sub(
                    dv[:, 3 * T:6 * T], xt[:, 0:3 * T], xt[:, 6 * T:9 * T])
                eng_d12 = nc.gpsimd if D12 == "pool" else nc.vector
                eng_d12.tensor_sub(
                    dv[:, 6 * T:9 * T], dv[:, 3 * T:6 * T], dv[:, 0:3 * T])

                sq = work.tile([P, 9, T], bf16, tag="sq")
                sv = sq[:].rearrange("p a b -> p (a b)")
                nd = SQ_DVE * T
                nc.vector.tensor_mul(sv[:, 0:nd], dv[:, 0:nd], dv[:, 0:nd])
                if SQ_DVE < 6:
                    nc.scalar.activation(
                        sv[:, nd:6 * T], dv[:, nd:6 * T], Act.Square)
                if SQ12 == "pool":
                    nc.gpsimd.tensor_mul(
                        sv[:, 6 * T:9 * T], dv[:, 6 * T:9 * T],
                        dv[:, 6 * T:9 * T])
                elif SQ12 == "dve":
                    nc.vector.tensor_mul(
                        sv[:, 6 * T:9 * T], dv[:, 6 * T:9 * T],
                        dv[:, 6 * T:9 * T])
                else:
                    nc.scalar.activation(
                        sv[:, 6 * T:9 * T], dv[:, 6 * T:9 * T], Act.Square)

                dt_ = dring.tile([P, 3, T], bf16)

                def root(dst, src, w):
                    """dst = sqrt(src): direct, or exp(0.5 ln) to stay in
                    the exp activation-table set."""
                    if NOSQRT:
                        lg = lgp.tile([P, w], bf16, tag=f"lg{w}")
                        nc.scalar.activation(lg[:], src, Act.Ln)
                        nc.scalar.activation(dst, lg[:], Act.Exp, scale=0.5)
                    else:
                        nc.scalar.activation(dst, src, Act.Sqrt)

                npe = 3 - SUMS_DVE  # pairs summed on PE (first npe pairs)
                if npe > 0:
                    q = psq.tile([P, npe, T], f32, tag="q")
                    for pi in range(npe):
                        for c in range(3):
                            nc.tensor.matmul(
                                q[:, pi, :], diag(_iWD_I()),
                                sq[:, 3 * pi + c, :],
                                start=(c == 0), stop=(c == 2))
                    root(dt_[:, 0:npe, :].rearrange("p a b -> p (a b)"),
                         q[:].rearrange("p a b -> p (a b)"), npe * T)
                if SUMS_DVE > 0:
                    qd = work.tile([P, SUMS_DVE, T], bf16, tag="qd")
                    for i in range(SUMS_DVE):
                        pi = npe + i
                        nc.vector.tensor_add(
                            qd[:, i, :], sq[:, 3 * pi, :], sq[:, 3 * pi + 1, :])
                        nc.vector.tensor_add(
                            qd[:, i, :], qd[:, i, :], sq[:, 3 * pi + 2, :])
                    root(dt_[:, npe:3, :].rearrange("p a b -> p (a b)"),
                         qd[:].rearrange("p a b -> p (a b)"), SUMS_DVE * T)
                return dt_

            def elu_tail(z, tag, ib, ieb):
                """max form: z holds v = W@x + b (bias PE-initialized);
                  h = elu(v) = max(min(exp(v),1)-1, v).
                relu form: z holds raw W@x; bias via scalars;
                  h = elu(v)+1 = relu(z+b) + min(exp(z)*e^b, 1)."""
                zf = z[:].rearrange("p a b -> p (a b)")
                e = mlp.tile([P, 2, T], bf16, tag=f"e{tag}")
                ef = e[:].rearrange("p a b -> p (a b)")
                nc.scalar.activation(ef, zf, Act.Exp)
                h = mlp.tile([P, 2, T], bf16, tag=f"h{tag}")
                if ELU == "max":
                    nc.vector.tensor_scalar(
                        out=ef, in0=ef, scalar1=1.0, scalar2=-1.0,
                        op0=Alu.min, op1=Alu.add)
                    nc.vector.tensor_tensor(
                        out=h[:].rearrange("p a b -> p (a b)"), in0=ef,
                        in1=zf, op=Alu.max)
                else:
                    r = mlp.tile([P, 2, T], bf16, tag=f"r{tag}")
                    for k in range(2):
                        nc.vector.tensor_scalar(
                            out=e[:, k, :], in0=e[:, k, :],
                            scalar1=ws(ieb(k)), scalar2=1.0,
                            op0=Alu.mult, op1=Alu.min)
                        nc.vector.tensor_scalar(
                            out=r[:, k, :], in0=z[:, k, :],
                            scalar1=ws(ib(k)), scalar2=0.0,
                            op0=Alu.add, op1=Alu.max)
                    nc.vector.tensor_add(
                        h[:].rearrange("p a b -> p (a b)"), ef,
                        r[:].rearrange("p a b -> p (a b)"))
                return h

            def emit_b(ti, dt_):
                """MLP from distances d [P,3,T]; writes Y tile."""
                src = 0 if bench_small else ti
                init = ELU == "max"
                z1 = pz1.tile([P, 2, T], f32, tag="z1")
                for k in range(2):
                    if init:
                        nc.tensor.matmul(
                            z1[:, k, :], diag(_iWD_I()),
                            bv[:, k * T:(k + 1) * T],
                            start=True, stop=False)
                    for j in range(3):
                        nc.tensor.matmul(
                            z1[:, k, :], diag(_iWD_W1(k, j)), dt_[:, j, :],
                            start=(not init and j == 0), stop=(j == 2))
                h1 = elu_tail(z1, "1", _ib1, _ieb1)

                z2 = pz.tile([P, 2, T], f32, tag="z2")
                for m in range(2):
                    if init:
                        nc.tensor.matmul(
                            z2[:, m, :], diag(_iWD_I()),
                            bv[:, (2 + m) * T:(3 + m) * T],
                            start=True, stop=False)
                    for j in range(2):
                        nc.tensor.matmul(
                            z2[:, m, :], diag(_iWD_W2(m, j)), h1[:, j, :],
                            start=(not init and j == 0), stop=(j == 1))
                h2 = elu_tail(z2, "2", _ib2x, _ieb2x)

                yz = pz.tile([P, T], f32, tag="yz")
                for j in range(2):
                    nc.tensor.matmul(
                        yz[:], diag(_iWD_W3(j)), h2[:, j, :],
                        start=(j == 0), stop=(j == 1))

                yr = Y[src * TILE:(src + 1) * TILE, :].rearrange(
                    "(p s) d -> p (s d)", p=P)
                yo = yout.tile([P, T], f32)
                yb = _IB3 if ELU == "max" else _IB3X
                if YOUT == "act":
                    nc.scalar.activation(
                        yo, yz[:], Act.Identity, bias=ws(yb), scale=1.0)
                else:
                    nc.vector.tensor_scalar(
                        out=yo, in0=yz[:], scalar1=ws(yb), scalar2=None,
                        op0=Alu.add)
                nc.sync.dma_start(out=yr, in_=yo[:])

            # reps>1 wraps the whole body in a For_i loop (benchmarking);
            # unroll=True python-unrolls instead (for the timeline sim,
            # which cannot resolve register branches)
            _loop = tc.For_i(0, reps) if reps != 1 and not unroll else None
            if _loop is not None:
                _loop.__enter__()

            n_groups = (N_TILES + TPG - 1) // TPG
            for _rep in range(reps if unroll else 1):
                for g in range(n_groups):
                    tis = range(g * TPG, min((g + 1) * TPG, N_TILES))
                    dts = [emit_a(ti) for ti in tis]
                    for ti, dt_ in zip(tis, dts):
                        emit_b(ti, dt_)

            if _loop is not None:
                _loop.__exit__(None, None, None)

    _split_sync_waits(nc, mybir, limit=1)
    return nc


def _pack_weights(W1, b1, W2, b2, W3, b3):
    import ml_dtypes
    W1 = np.asarray(W1, np.float32); b1 = np.asarray(b1, np.float32)
    W2 = np.asarray(W2, np.float32); b2 = np.asarray(b2, np.float32)
    W3 = np.asarray(W3, np.float32); b3 = np.asarray(b3, np.float32)
    b2x = b2 - W2.sum(axis=1)
    b3x = b3 - W3.sum(axis=1)
    wb = np.empty(N_WB, np.float32)
    wb[_IB3] = b3[0]
    wb[_IB3X] = b3x[0]
    wb[_ib1(0)], wb[_ib1(1)] = b1
    wb[_ib2x(0)], wb[_ib2x(1)] = b2x
    wb[_ieb1(0)], wb[_ieb1(1)] = np.exp(b1)
    wb[_ieb2x(0)], wb[_ieb2x(1)] = np.exp(b2x)

    bf = ml_dtypes.bfloat16
    eye = np.eye(P, dtype=np.float32)
    wdf = np.empty((N_WD, P, P), np.float32)
    wdf[_iWD_I()] = eye
    for k in range(2):
        for j in range(3):
            wdf[_iWD_W1(k, j)] = eye * W1[k, j]
    for m in range(2):
        for j in range(2):
            wdf[_iWD_W2(m, j)] = eye * W2[m, j]
    for j in range(2):
        wdf[_iWD_W3(j)] = eye * W3[0, j]
    bvec = np.empty(4 * T, np.float32)
    bvec[0:T] = b1[0]
    bvec[T:2 * T] = b1[1]
    bvec[2 * T:3 * T] = b2[0]
    bvec[3 * T:4 * T] = b2[1]
    return wb, wdf.astype(bf), bvec.astype(bf)


def _pack_x(X):
    """[B,3,3] fp32 -> per-core bf16 plane-major [N_TILES*P, 9*T]."""
    import ml_dtypes
    Xb = np.asarray(X, np.float32).reshape(B, 9).astype(ml_dtypes.bfloat16)
    out = []
    for c in range(N_CORES):
        xc = Xb[c * B_CORE:(c + 1) * B_CORE]
        xc = xc.reshape(N_TILES, P, T, 9).transpose(0, 1, 3, 2)
        out.append(np.ascontiguousarray(xc).reshape(N_TILES * P, 9 * T))
    return out


LAST_RESULTS = None  # BassKernelResults of the most recent run (for test.py)


def kernel(X, W1, b1, W2, b2, W3, b3):
    from concourse.bass_utils import run_bass_kernel_spmd
    global LAST_RESULTS

    xs = _pack_x(X)
    wb, wd, bvec = _pack_weights(W1, b1, W2, b2, W3, b3)

    if "nc" not in _CACHE:
        _CACHE["nc"] = _build()
    nc = _CACHE["nc"]

    in_maps = [
        {"X": xs[c], "WB": wb, "WD": wd, "BV": bvec}
        for c in range(N_CORES)
    ]
    res = run_bass_kernel_spmd(nc, in_maps, core_ids=list(range(N_CORES)))
    LAST_RESULTS = res
    out = np.concatenate([res.results[c]["Y"] for c in range(N_CORES)], axis=0)
    return out.reshape(B, 1)
